# revision 1
# baseline (speedup 1.0000x reference)
"""Trainium2 Bass kernel for nn_DCNO_85813446574233.

Data-parallel over batch: 8 samples -> 8 NeuronCores, one sample per core.
Per-core layout: channel-major [C=128 partitions, H*W=16384 free].

Phases per sample:
  P0  load x, time-MLPs (PE), GN1 stats
  P1  AdaGN1+spectral conv (GN affine folded into conv weights) -> silu -> +res
  P2  AdaGN2+windowed attention (S^T block-diag formulation)    -> +res
  P3  AdaGN3+channel MLP                                        -> +res
  P4  GN4 -> depthwise 3x3 (9 diagonal matmuls, clipped APs) -> silu -> pw -> +res
"""

import numpy as np

B = 8
C = 128
H = W = 128
S = H * W            # 16384
HEADS = 8
HD = 16
WS = 4
T = WS * WS          # 16 tokens/window
GROUPS = 32
EPS = 1e-5
NW_X = W // WS       # 32 windows per row
NW = NW_X * (H // WS)  # 1024
CHUNK = 512          # spatial chunk = one window row = 32 windows
NCH = S // CHUNK     # 32
AW = 32              # windows per attention chunk

_BUILT = None


def _rel_pos_index():
    c = np.stack(np.meshgrid(np.arange(WS), np.arange(WS), indexing="ij")).reshape(2, -1)
    rel = (c[:, :, None] - c[:, None, :]).transpose(1, 2, 0)
    rel[:, :, 0] += WS - 1
    rel[:, :, 1] += WS - 1
    rel[:, :, 0] *= 2 * WS - 1
    return rel.sum(-1)  # (16, 16) [t, s']


def _build_bass():
    import concourse.bacc as bacc
    import concourse.mybir as mybir
    import concourse.tile as tile

    TC = tile.TileContext

    f32 = mybir.dt.float32
    bf16 = mybir.dt.bfloat16
    AF = mybir.ActivationFunctionType
    OP = mybir.AluOpType
    AX = mybir.AxisListType

    nc = bacc.Bacc()

    def din(name, shape, dt=f32):
        return nc.declare_dram_parameter(name, list(shape), dt, isOutput=False)

    x_in = din("x_in", [C, S])
    temb = din("temb", [C, 2])
    tm_w1 = din("tm_w1", [3, 2, C, 4 * C])
    tm_b1 = din("tm_b1", [C, 12])
    tm_w2 = din("tm_w2", [3, 4, C, 2 * C])
    tm_b2 = din("tm_b2", [C, 6])
    gn_w = din("gn_w", [C, 4])
    gn_b = din("gn_b", [C, 4])
    mem_fw = din("mem_fw", [C, GROUPS])
    mem_bw = din("mem_bw", [GROUPS, C])
    spec_wT = din("spec_wT", [C, C])
    spec_b = din("spec_b", [C, 1])
    qkv_wT = din("qkv_wT", [C, 3 * C])
    qkv_b = din("qkv_b", [C, 3])
    proj_wT = din("proj_wT", [C, C], bf16)
    proj_b = din("proj_b", [C, 1])
    mlp1_wT = din("mlp1_wT", [C, 4 * C])
    mlp1_b = din("mlp1_b", [C, 4])
    mlp2_wT = din("mlp2_wT", [4, C, C], bf16)
    mlp2_b = din("mlp2_b", [C, 1])
    pw_wT = din("pw_wT", [C, C], bf16)
    pw_b = din("pw_b", [C, 1])
    dw_diag = din("dw_diag", [9, C, C], bf16)
    dw_b = din("dw_b", [C, 1])
    ebias = din("ebias", [C, AW * T], bf16)
    rmat = din("rmat", [C, C], bf16)
    ipad = din("ipad", [C, HD], bf16)               # ipad[(h,d), j] = (d == j)
    out_d = nc.declare_dram_parameter("out", [C, S], f32, isOutput=True)

    with TC(nc) as tc:
        with (
            tc.tile_pool(name="big", bufs=1) as big,
            tc.tile_pool(name="wp", bufs=1) as wp,
            tc.tile_pool(name="cp", bufs=1) as cp,
            tc.tile_pool(name="psmm", bufs=2, space="PSUM") as psmm,
            tc.tile_pool(name="psatt", bufs=4, space="PSUM") as psatt,
            tc.tile_pool(name="pssm", bufs=2, space="PSUM") as pssm,
        ):
            dma = nc.sync.dma_start

            _tc = [0]

            def mmtile():
                _tc[0] += 1
                return psmm.tile([C, CHUNK], f32, tag="mm", name=f"mm{_tc[0]}")

            def atttile():
                _tc[0] += 1
                return psatt.tile([C, CHUNK], f32, tag="att", name=f"att{_tc[0]}")

            def smtile(p=C, n=16):
                _tc[0] += 1
                return pssm.tile([p, n], f32, tag="sm", name=f"sm{_tc[0]}")

            # ---------------- persistent SBUF ----------------
            xres = big.tile([C, S], f32)
            HP = H + 2
            WP = W + 2
            hbuf = big.tile([C, HP * WP], bf16)  # zero-padded for dw conv
            dma(xres[:], x_in[:])

            w_spec = wp.tile([C, C], f32);      dma(w_spec[:], spec_wT[:])
            w_qkv = wp.tile([C, 3 * C], f32);   dma(w_qkv[:], qkv_wT[:])
            w_proj = wp.tile([C, C], bf16);     dma(w_proj[:], proj_wT[:])
            w_mlp1 = wp.tile([C, 4 * C], f32);  dma(w_mlp1[:], mlp1_wT[:])
            w_mlp2 = wp.tile([C, 4 * C], bf16)
            for k in range(4):
                dma(w_mlp2[:, k * C:(k + 1) * C], mlp2_wT[k])
            w_pw = wp.tile([C, C], bf16);       dma(w_pw[:], pw_wT[:])
            w_dw = wp.tile([C, 9 * C], bf16)
            for k in range(9):
                dma(w_dw[:, k * C:(k + 1) * C], dw_diag[k])
            w_memf = wp.tile([C, GROUPS], f32); dma(w_memf[:], mem_fw[:])
            w_memb = wp.tile([GROUPS, C], f32); dma(w_memb[:], mem_bw[:])
            w_eb = wp.tile([C, AW * T], bf16);  dma(w_eb[:], ebias[:])
            w_r = wp.tile([C, C], bf16);        dma(w_r[:], rmat[:])
            w_ip = wp.tile([C, HD], bf16);      dma(w_ip[:], ipad[:])
            v_gnw = wp.tile([C, 4], f32);       dma(v_gnw[:], gn_w[:])
            v_gnb = wp.tile([C, 4], f32);       dma(v_gnb[:], gn_b[:])
            v_specb = wp.tile([C, 1], f32);     dma(v_specb[:], spec_b[:])
            v_qkvb = wp.tile([C, 3], f32);      dma(v_qkvb[:], qkv_b[:])
            v_projb = wp.tile([C, 1], f32);     dma(v_projb[:], proj_b[:])
            v_mlp1b = wp.tile([C, 4], f32);     dma(v_mlp1b[:], mlp1_b[:])
            v_mlp2b = wp.tile([C, 1], f32);     dma(v_mlp2b[:], mlp2_b[:])
            v_pwb = wp.tile([C, 1], f32);       dma(v_pwb[:], pw_b[:])
            v_dwb = wp.tile([C, 1], f32);       dma(v_dwb[:], dw_b[:])
            t_temb = wp.tile([C, 2], f32);      dma(t_temb[:], temb[:])

            sshift = cp.tile([C, 6], f32)       # s1,sh1,s2,sh2,s3,sh3
            eps_t = cp.tile([C, 1], f32)
            nc.vector.memset(eps_t[:], EPS)

            # ---------------- time MLPs (scoped pool, freed after) ----------------
            with tc.tile_pool(name="tmw", bufs=1) as tmw:
                t_tmw1 = tmw.tile([C, 3 * 2 * 4 * C], f32)
                t_tmw2 = tmw.tile([C, 3 * 4 * 2 * C], f32)
                for m in range(3):
                    for k in range(2):
                        dma(t_tmw1[:, (m * 2 + k) * 4 * C:(m * 2 + k + 1) * 4 * C], tm_w1[m, k])
                    for k in range(4):
                        dma(t_tmw2[:, (m * 4 + k) * 2 * C:(m * 4 + k + 1) * 2 * C], tm_w2[m, k])
                t_tmb1 = tmw.tile([C, 12], f32); dma(t_tmb1[:], tm_b1[:])
                t_tmb2 = tmw.tile([C, 6], f32);  dma(t_tmb2[:], tm_b2[:])

                p_h1 = smtile()
                for m in range(3):
                    for mc in range(4):
                        col = m * 4 + mc
                        for k in range(2):
                            lhs = t_tmw1[:, ((m * 2 + k) * 4 + mc) * C:((m * 2 + k) * 4 + mc + 1) * C]
                            nc.tensor.matmul(p_h1[:, col:col + 1], lhs, t_temb[:, k:k + 1],
                                             start=(k == 0), stop=(k == 1))
                tmh = cp.tile([C, 12], f32)
                tmh_pre = cp.tile([C, 12], f32)
                nc.vector.tensor_add(tmh_pre[:], p_h1[:, :12], t_tmb1[:])
                nc.scalar.activation(tmh[:], tmh_pre[:], AF.Silu)
                p_ss = smtile()
                for m in range(3):
                    for j in range(2):
                        col = m * 2 + j
                        for k in range(4):
                            lhs = t_tmw2[:, ((m * 4 + k) * 2 + j) * C:((m * 4 + k) * 2 + j + 1) * C]
                            nc.tensor.matmul(p_ss[:, col:col + 1], lhs, tmh[:, m * 4 + k:m * 4 + k + 1],
                                             start=(k == 0), stop=(k == 3))
                nc.vector.tensor_add(sshift[:], p_ss[:, :6], t_tmb2[:])

            # ---------------- GN helpers ----------------
            inv_n = 1.0 / (S * (C // GROUPS))

            def gn_coefs(gi, sum_col, sq_col, tpair):
                st = cp.tile([C, 2], f32, tag=f"st{gi}")
                nc.vector.tensor_copy(st[:, 0:1], sum_col)
                nc.vector.tensor_copy(st[:, 1:2], sq_col)
                pg = smtile(GROUPS, 2)
                nc.tensor.matmul(pg[:], w_memf[:], st[:])
                sg = cp.tile([GROUPS, 2], f32, tag=f"sg{gi}")
                nc.vector.tensor_copy(sg[:], pg[:])
                pb = smtile(C, 2)
                nc.tensor.matmul(pb[:], w_memb[:], sg[:])
                mean = cp.tile([C, 1], f32, tag=f"mean{gi}")
                var = cp.tile([C, 1], f32, tag=f"var{gi}")
                nc.scalar.mul(mean[:], pb[:, 0:1], inv_n)
                m2 = cp.tile([C, 1], f32, tag=f"m2{gi}")
                nc.vector.tensor_mul(m2[:], mean[:], mean[:])
                nc.vector.scalar_tensor_tensor(var[:], pb[:, 1:2], inv_n, m2[:],
                                               op0=OP.mult, op1=OP.subtract)
                lnv = cp.tile([C, 1], f32, tag=f"lnv{gi}")
                nc.scalar.activation(lnv[:], var[:], AF.Ln, bias=eps_t[:])
                rstd = cp.tile([C, 1], f32, tag=f"rstd{gi}")
                nc.scalar.activation(rstd[:], lnv[:], AF.Exp, scale=-0.5)
                a = cp.tile([C, 1], f32, tag=f"a{gi}")
                b = cp.tile([C, 1], f32, tag=f"b{gi}")
                if tpair is not None:
                    si, shi = tpair
                    t1 = cp.tile([C, 1], f32, tag=f"t1{gi}")
                    nc.vector.tensor_scalar_add(t1[:], sshift[:, si:si + 1], 1.0)
                    aw_ = cp.tile([C, 1], f32, tag=f"aw{gi}")
                    nc.vector.tensor_mul(aw_[:], rstd[:], v_gnw[:, gi:gi + 1])
                    nc.vector.tensor_mul(a[:], aw_[:], t1[:])
                    u = cp.tile([C, 1], f32, tag=f"u{gi}")
                    nc.vector.tensor_mul(u[:], mean[:], a[:])
                    v2 = cp.tile([C, 1], f32, tag=f"v2{gi}")
                    nc.vector.scalar_tensor_tensor(v2[:], v_gnb[:, gi:gi + 1], t1[:], u[:],
                                                   op0=OP.mult, op1=OP.subtract)
                    nc.vector.tensor_add(b[:], v2[:], sshift[:, shi:shi + 1])
                else:
                    nc.vector.tensor_mul(a[:], rstd[:], v_gnw[:, gi:gi + 1])
                    u = cp.tile([C, 1], f32, tag=f"u{gi}")
                    nc.vector.tensor_mul(u[:], mean[:], a[:])
                    nc.vector.tensor_sub(b[:], v_gnb[:, gi:gi + 1], u[:])
                return a, b

            def fold_weight(wt_ap, a, ncols, tag):
                weff = wp.tile([C, ncols], bf16, tag=tag)
                nc.vector.tensor_scalar_mul(weff[:], wt_ap, a[:])
                return weff

            def fold_bias(wt_ap, b, bias_ap, tag):
                """beff[o,1] = sum_c wt[c,o]*b[c] + bias[o]   (128 cols)"""
                pbias = smtile(C, 1)
                nc.tensor.matmul(pbias[:, 0:1], wt_ap, b[:])
                beff = cp.tile([C, 1], f32, tag=tag)
                nc.vector.tensor_add(beff[:], pbias[:, 0:1], bias_ap)
                return beff

            # ---------------- GN1 stats ----------------
            g1sum = cp.tile([C, 1], f32)
            g1sq = cp.tile([C, 1], f32)
            nc.vector.tensor_reduce(g1sum[:], xres[:], axis=AX.X, op=OP.add)
            nc.scalar.activation(hbuf[:, :S], xres[:], AF.Square, accum_out=g1sq[:])
            a1, b1 = gn_coefs(0, g1sum[:], g1sq[:], (0, 1))
            w_spec_e = fold_weight(w_spec[:], a1, C, "wspece")
            b_spec_e = fold_bias(w_spec[:], b1, v_specb[:], "bspece")

            with (
                tc.tile_pool(name="work", bufs=2) as work,
                tc.tile_pool(name="attw", bufs=2) as attw,
            ):
                # ---------------- P1: spectral ----------------
                g2sum = cp.tile([C, NCH], f32)
                g2sq = cp.tile([C, NCH], f32)
                for i in range(NCH):
                    sl = slice(i * CHUNK, (i + 1) * CHUNK)
                    xc = work.tile([C, CHUNK], bf16, tag="xc")
                    nc.vector.tensor_copy(xc[:], xres[:, sl])
                    pconv = mmtile()
                    nc.tensor.matmul(pconv[:], w_spec_e[:], xc[:])
                    hs = work.tile([C, CHUNK], bf16, tag="hs")
                    nc.scalar.activation(hs[:], pconv[:], AF.Silu, bias=b_spec_e[:])
                    nc.vector.scalar_tensor_tensor(xres[:, sl], hs[:], 0.0, xres[:, sl],
                                                   op0=OP.add, op1=OP.add,
                                                   accum_out=g2sum[:, i:i + 1])
                    sq = work.tile([C, CHUNK], f32, tag="sqd")
                    nc.scalar.activation(sq[:], xres[:, sl], AF.Square,
                                         accum_out=g2sq[:, i:i + 1])

                g2sum1 = cp.tile([C, 1], f32)
                g2sq1 = cp.tile([C, 1], f32)
                nc.vector.tensor_reduce(g2sum1[:], g2sum[:], axis=AX.X, op=OP.add)
                nc.vector.tensor_reduce(g2sq1[:], g2sq[:], axis=AX.X, op=OP.add)
                a2, b2 = gn_coefs(1, g2sum1[:], g2sq1[:], (2, 3))
                w_qkv_e = fold_weight(w_qkv[:], a2, 3 * C, "wqkve")
                b_qkv = [fold_bias(w_qkv[:, g * C:(g + 1) * C], b2,
                                   v_qkvb[:, g:g + 1], f"bqkv{g}") for g in range(3)]

                # ---------------- P2: attention ----------------
                # double-buffered block-diag tiles (A/B alternate per chunk)
                bd_kT = [big.tile([C, AW * C], bf16, tag=f"bdk{j}", name=f"bdk{j}") for j in range(2)]
                bd_vT = [big.tile([C, AW * C], bf16, tag=f"bdv{j}", name=f"bdv{j}") for j in range(2)]
                bd_wT = [big.tile([C, AW * C], bf16, tag=f"bdw{j}", name=f"bdw{j}") for j in range(2)]
                for j in range(2):
                    nc.vector.memset(bd_kT[j][:], 0.0)
                    nc.vector.memset(bd_vT[j][:], 0.0)
                    nc.vector.memset(bd_wT[j][:], 0.0)
                g3sum = cp.tile([C, NCH * WS], f32)
                g3sq = cp.tile([C, NCH], f32)

                for i in range(NCH):
                    sl = slice(i * CHUNK, (i + 1) * CHUNK)
                    bd_k = bd_kT[i % 2]
                    bd_v = bd_vT[i % 2]
                    bd_w = bd_wT[i % 2]
                    xc2 = attw.tile([C, CHUNK], bf16, tag="xc2")
                    nc.vector.tensor_copy(xc2[:], xres[:, sl])
                    qg = attw.tile([C, CHUNK], bf16, tag="qg")
                    kg = attw.tile([C, CHUNK], bf16, tag="kg")
                    vg = attw.tile([C, CHUNK], bf16, tag="vg")
                    for g, dst in ((0, qg), (1, kg), (2, vg)):
                        pq = mmtile()
                        nc.tensor.matmul(pq[:], w_qkv_e[:, g * C:(g + 1) * C], xc2[:])
                        d3 = dst[:].rearrange("c (w ty tx) -> c ty w tx", w=NW_X, ty=WS, tx=WS)
                        p3 = pq[:].rearrange("c (ty w tx) -> c ty w tx", ty=WS, w=NW_X, tx=WS)
                        for ty in range(WS):
                            nc.vector.tensor_scalar_add(d3[:, ty], p3[:, ty], b_qkv[g][:])
                    # block-diag builds: k/w via HWDGE (sync+scalar), v via
                    # gpsimd SWDGE (independent descriptor-gen path)
                    dmae = [nc.sync, nc.scalar]
                    for h in range(HEADS):
                        hp = slice(h * HD, (h + 1) * HD)
                        bk_dst = bd_k[hp].rearrange("p (w x) -> p w x", w=AW, x=C)[:, :, h * T:(h + 1) * T]
                        bv_dst = bd_v[hp].rearrange("p (w x) -> p w x", w=AW, x=C)[:, :, h * T:(h + 1) * T]
                        dmae[h % 2].dma_start(bk_dst, kg[hp].rearrange("p (w t) -> p w t", w=AW, t=T))
                        nc.gpsimd.dma_start(bv_dst, vg[hp].rearrange("p (w t) -> p w t", w=AW, t=T))
                    # scores S^T[(h,s'),(w,t)] and v^T[(h,s'),(w,d)]
                    pst = atttile()
                    pvt = atttile()
                    for w in range(AW):
                        nc.tensor.matmul(pst[:, w * T:(w + 1) * T],
                                         bd_k[:, w * C:(w + 1) * C],
                                         qg[:, w * T:(w + 1) * T])
                        nc.tensor.matmul(pvt[:, w * HD:(w + 1) * HD],
                                         bd_v[:, w * C:(w + 1) * C],
                                         w_ip[:])
                    pt = attw.tile([C, CHUNK], bf16, tag="pt")
                    nc.scalar.activation(pt[:], pst[:], AF.Exp)
                    nc.vector.tensor_mul(pt[:], pt[:], w_eb[:])
                    vt = attw.tile([C, CHUNK], bf16, tag="vt")
                    nc.scalar.copy(vt[:], pvt[:])
                    for h in range(HEADS):
                        hp = slice(h * HD, (h + 1) * HD)
                        bw_dst = bd_w[hp].rearrange("p (w x) -> p w x", w=AW, x=C)[:, :, h * HD:(h + 1) * HD]
                        dmae[h % 2].dma_start(bw_dst, vt[hp].rearrange("p (w d) -> p w d", w=AW, d=HD))
                    pden = atttile()
                    nc.tensor.matmul(pden[:], w_r[:], pt[:])
                    rden = attw.tile([C, CHUNK], f32, tag="rden")
                    nc.vector.reciprocal_approx_fast(rden[:], pden[:])
                    pav = atttile()
                    for w in range(AW):
                        nc.tensor.matmul(pav[:, w * T:(w + 1) * T],
                                         bd_w[:, w * C:(w + 1) * C],
                                         pt[:, w * T:(w + 1) * T])
                    hat = attw.tile([C, CHUNK], bf16, tag="hat")
                    nc.vector.tensor_mul(hat[:], pav[:], rden[:])
                    ppr = mmtile()
                    nc.tensor.matmul(ppr[:], w_proj[:], hat[:])
                    xr_ap = xres[:, sl].rearrange("c (ty w tx) -> c ty w tx", ty=WS, w=NW_X, tx=WS)
                    pr_ap = ppr[:].rearrange("c (w ty tx) -> c ty w tx", w=NW_X, ty=WS, tx=WS)
                    for ty in range(WS):
                        nc.vector.scalar_tensor_tensor(xr_ap[:, ty], pr_ap[:, ty], v_projb[:],
                                                       xr_ap[:, ty],
                                                       op0=OP.add, op1=OP.add,
                                                       accum_out=g3sum[:, i * WS + ty:i * WS + ty + 1])
                    sq3 = work.tile([C, CHUNK], f32, tag="sqd")
                    nc.scalar.activation(sq3[:], xres[:, sl], AF.Square,
                                         accum_out=g3sq[:, i:i + 1])

                g3sum1 = cp.tile([C, 1], f32)
                g3sq1 = cp.tile([C, 1], f32)
                nc.vector.tensor_reduce(g3sum1[:], g3sum[:], axis=AX.X, op=OP.add)
                nc.vector.tensor_reduce(g3sq1[:], g3sq[:], axis=AX.X, op=OP.add)
                a3, b3 = gn_coefs(2, g3sum1[:], g3sq1[:], (4, 5))
                w_mlp1_e = fold_weight(w_mlp1[:], a3, 4 * C, "wmlp1e")
                b_mlp1 = [fold_bias(w_mlp1[:, g * C:(g + 1) * C], b3,
                                    v_mlp1b[:, g:g + 1], f"bmlp1{g}") for g in range(4)]

                # ---------------- P3: channel MLP ----------------
                g4sum = cp.tile([C, NCH], f32)
                g4sq = cp.tile([C, NCH], f32)
                for i in range(NCH):
                    sl = slice(i * CHUNK, (i + 1) * CHUNK)
                    xc3 = work.tile([C, CHUNK], bf16, tag="xc")
                    nc.vector.tensor_copy(xc3[:], xres[:, sl])
                    h1 = work.tile([C, 4 * CHUNK], bf16, tag="p3h1")
                    for g in range(4):
                        pm = mmtile()
                        nc.tensor.matmul(pm[:], w_mlp1_e[:, g * C:(g + 1) * C], xc3[:])
                        nc.scalar.activation(h1[:, g * CHUNK:(g + 1) * CHUNK], pm[:],
                                             AF.Silu, bias=b_mlp1[g][:])
                    pm2 = mmtile()
                    for g in range(4):
                        nc.tensor.matmul(pm2[:], w_mlp2[:, g * C:(g + 1) * C],
                                         h1[:, g * CHUNK:(g + 1) * CHUNK],
                                         start=(g == 0), stop=(g == 3))
                    nc.vector.scalar_tensor_tensor(xres[:, sl], pm2[:], v_mlp2b[:], xres[:, sl],
                                                   op0=OP.add, op1=OP.add,
                                                   accum_out=g4sum[:, i:i + 1])
                    sq4 = work.tile([C, CHUNK], f32, tag="sqd")
                    nc.scalar.activation(sq4[:], xres[:, sl], AF.Square,
                                         accum_out=g4sq[:, i:i + 1])

                g4sum1 = cp.tile([C, 1], f32)
                g4sq1 = cp.tile([C, 1], f32)
                nc.vector.tensor_reduce(g4sum1[:], g4sum[:], axis=AX.X, op=OP.add)
                nc.vector.tensor_reduce(g4sq1[:], g4sq[:], axis=AX.X, op=OP.add)
                a4, b4 = gn_coefs(3, g4sum1[:], g4sq1[:], None)

                # ---------------- P4: spatial block ----------------
                # zero the pad border (rows 0/129, cols 0/129 of 130x130)
                hbp = hbuf[:].rearrange("c (y x) -> c y x", y=HP, x=WP)
                nc.vector.memset(hbp[:, 0, :], 0.0)
                nc.vector.memset(hbp[:, HP - 1, :], 0.0)
                nc.vector.memset(hbp[:, :, 0], 0.0)
                nc.vector.memset(hbp[:, :, WP - 1], 0.0)
                # normalized input written to interior
                xr3 = xres[:].rearrange("c (y x) -> c y x", y=H, x=W)
                nc.vector.tensor_scalar(hbp[:, 1:H + 1, 1:W + 1], xr3, a4[:], b4[:],
                                        op0=OP.mult, op1=OP.add)
                for i in range(NCH):
                    y0 = i * 4
                    pdw = mmtile()
                    for k in range(9):
                        dy, dx = k // 3 - 1, k % 3 - 1
                        in_ap = hbp[:, y0 + 1 + dy:y0 + 5 + dy, 1 + dx:W + 1 + dx]
                        nc.tensor.matmul(pdw[:], w_dw[:, k * C:(k + 1) * C], in_ap,
                                         start=(k == 0), stop=(k == 8))
                    hdw = work.tile([C, CHUNK], bf16, tag="hs")
                    nc.scalar.activation(hdw[:], pdw[:], AF.Silu, bias=v_dwb[:])
                    ppw = mmtile()
                    nc.tensor.matmul(ppw[:], w_pw[:], hdw[:])
                    sl = slice(i * CHUNK, (i + 1) * CHUNK)
                    nc.vector.scalar_tensor_tensor(xres[:, sl], ppw[:], v_pwb[:], xres[:, sl],
                                                   op0=OP.add, op1=OP.add)
                    dma(out_d[:, sl], xres[:, sl])

    nc.compile()
    return nc


def _prepare_inputs(inputs):
    import ml_dtypes
    f = lambda k: np.asarray(inputs[k], dtype=np.float32)

    x = f("x")
    t_emb = f("t_emb")

    sig_mode = 1.0 / (1.0 + np.exp(-f("mode_w")))
    spec_wT = (f("spec_w") * sig_mode[None, :]).T.copy()

    qscale = HD ** -0.5
    qkv_wT = f("qkv_w").T.copy()
    qkv_wT[:, :C] *= qscale
    qkv_b = f("qkv_b")
    qkv_bc = np.stack([qkv_b[:C] * qscale, qkv_b[C:2 * C], qkv_b[2 * C:]], axis=1).copy()

    rel = _rel_pos_index()
    bias = f("rel_bias")[rel]                       # (16 t, 16 s', 8 h)
    ebias_flat = np.exp(bias.transpose(2, 1, 0)).reshape(C, T)  # [(h,s'), t]
    ebias_tiled = np.concatenate([ebias_flat] * AW, axis=1)

    rmat = np.zeros((C, C), dtype=np.float32)
    for h in range(HEADS):
        rmat[h * T:(h + 1) * T, h * T:(h + 1) * T] = 1.0

    mem_fw = np.zeros((C, GROUPS), dtype=np.float32)
    for c in range(C):
        mem_fw[c, c // (C // GROUPS)] = 1.0

    dw_w = f("dw_w")
    dw_diag = np.zeros((9, C, C), dtype=np.float32)
    for k in range(9):
        np.fill_diagonal(dw_diag[k], dw_w[:, 0, k // 3, k % 3])

    tm_w1 = np.stack([
        np.stack([f(f"t{i}_w1").T[:C], f(f"t{i}_w1").T[C:]]) for i in (1, 2, 3)
    ]).copy()
    tm_w2 = np.stack([
        np.stack([f(f"t{i}_w2").T[k * C:(k + 1) * C] for k in range(4)]) for i in (1, 2, 3)
    ]).copy()
    tm_b1 = np.zeros((C, 12), dtype=np.float32)
    tm_b2 = np.zeros((C, 6), dtype=np.float32)
    for m, i in enumerate((1, 2, 3)):
        b1_ = f(f"t{i}_b1")
        for mc in range(4):
            tm_b1[:, m * 4 + mc] = b1_[mc * C:(mc + 1) * C]
        b2_ = f(f"t{i}_b2")
        tm_b2[:, m * 2 + 0] = b2_[:C]
        tm_b2[:, m * 2 + 1] = b2_[C:]

    gn_w = np.stack([f(k) for k in ("gn1_w", "gn2_w", "gn3_w", "sb_gn_w")], axis=1).copy()
    gn_b = np.stack([f(k) for k in ("gn1_b", "gn2_b", "gn3_b", "sb_gn_b")], axis=1).copy()

    mlp2_wT = f("mlp_w2").T.copy()
    mlp2_slices = np.stack([mlp2_wT[k * C:(k + 1) * C] for k in range(4)]).copy()

    bf = ml_dtypes.bfloat16
    shared = {
        "tm_w1": tm_w1, "tm_b1": tm_b1, "tm_w2": tm_w2, "tm_b2": tm_b2,
        "gn_w": gn_w, "gn_b": gn_b, "mem_fw": mem_fw, "mem_bw": mem_fw.T.copy(),
        "spec_wT": spec_wT, "spec_b": f("spec_b").reshape(C, 1),
        "qkv_wT": qkv_wT, "qkv_b": qkv_bc,
        "proj_wT": f("proj_w").T.copy().astype(bf),
        "proj_b": f("proj_b").reshape(C, 1),
        "mlp1_wT": f("mlp_w1").T.copy(),
        "mlp1_b": f("mlp_b1").reshape(4, C).T.copy(),
        "mlp2_wT": mlp2_slices.astype(bf),
        "mlp2_b": f("mlp_b2").reshape(C, 1),
        "pw_wT": f("pw_w").T.copy().astype(bf),
        "pw_b": f("pw_b").reshape(C, 1),
        "dw_diag": dw_diag.astype(bf),
        "dw_b": f("dw_b").reshape(C, 1),
        "ebias": ebias_tiled.astype(bf), "rmat": rmat.astype(bf),
        "ipad": np.concatenate([np.eye(HD, dtype=np.float32)] * HEADS, axis=0).astype(bf),
    }
    in_maps = []
    for i in range(B):
        m = dict(shared)
        m["x_in"] = x[i].reshape(C, S).copy()
        m["temb"] = t_emb[i].reshape(2, C).T.copy()
        in_maps.append(m)
    return in_maps


def kernel(**inputs):
    global _BUILT
    from concourse.bass_utils import run_bass_kernel_spmd
    if _BUILT is None:
        _BUILT = _build_bass()
    nc = _BUILT
    in_maps = _prepare_inputs(inputs)
    res = run_bass_kernel_spmd(nc, in_maps, list(range(B)))
    out = np.stack([np.asarray(res.results[i]["out"]).reshape(C, H, W) for i in range(B)])
    return out.astype(np.float32)

